# revision 19
# baseline (speedup 1.0000x reference)
"""Llama attention layer (B=2, S=2048, D=2048, H=16, DH=128) on 8 TRN2 NeuronCores.

Sharding: 2-way data parallel over batch x 4-way tensor parallel over heads.
Core c: batch g = c // 4, heads 4r..4r+3 where r = c % 4.
Projections are column-parallel (each core computes Q/K/V for its 4 heads),
attention is fully local per (batch, head), then the per-head attention
outputs (kept transposed, [dim, seq]) are AllGather'd within each 4-core
batch group, and o_proj is column-parallel: core c computes output columns
r*512..(r+1)*512 of its batch. Host concatenates - no host-side compute.

Schedule: attention blocks run in ascending order; their outputs are
gathered with just TWO AllGathers (blocks {0,1} and {2,3}) because
collectives serialize on the issuing engine - with four gathers the chain
(25-45us per op, straggler-bound) finished ~40us after the last o_proj
consumer needed it. AG{01} fires at attn1-end and AG{23} at attn3-end, both
input-bound; o_proj runs at the end of the PE stream consuming blocks in
order 0..3, so only o_proj2 can briefly wait. All inputs are packed
partition-major on the host so DMA lines are 4KB+, spread over three DMA
queues (early DMA is descriptor-rate limited per queue).

All matmul operands are bf16 (fp32 accumulation in PSUM); softmax runs
without max-subtraction (scores are O(6), exp is safe in fp32); the
denominator is accumulated on the vector engine in bf16 and reduced over
partitions with a single ones-matmul per (head, seq-block), deferred into
the next head's stream so the PE never waits on the trailing exp chain. The
attention inner loop is software-pipelined: pair p+1's QK matmuls are
emitted before pair p's AV matmuls, hiding the scalar-engine exp latency.
"""

import os
import sys

for _p in ("/opt/trn_rl_repo", "/root/.axon_site/_ro/trn_rl_repo"):
    if os.path.isdir(_p) and _p not in sys.path:
        sys.path.append(_p)

import numpy as np
import ml_dtypes

import concourse.bass as bass
import concourse.tile as tile
import concourse.mybir as mybir
from concourse import bacc
from concourse.bass_utils import run_bass_kernel_spmd

F32 = mybir.dt.float32
BF16 = mybir.dt.bfloat16
AF = mybir.ActivationFunctionType

B, S, D, H, DH = 2, 2048, 2048, 16, 128
NCORES = 8
TP = 4                 # cores per batch group
HPC = H // TP          # heads per core = 4
SBLK = 512             # seq block (matmul moving size)
NSB = S // SBLK        # 4
DTILES = D // 128      # 16 contraction tiles
KT = S // 128          # 16 key tiles
OCOLS = D // TP        # 512 output columns per core
SCALE = 1.0 / float(np.sqrt(DH))
DMA_SPLIT = 4          # split big input DMAs so compute starts early
CSTEP = DTILES // DMA_SPLIT
RG = [[0, 1, 2, 3], [4, 5, 6, 7]]

DT = BF16              # matmul operand dtype
NPDT = ml_dtypes.bfloat16


def _emit(tc):
    nc = tc.nc
    # Inputs packed partition-major on host: per partition line contiguous.
    xP = nc.dram_tensor("xP", [128, NSB * DTILES, SBLK], DT,
                        kind="ExternalInput").ap()
    wqP = nc.dram_tensor("wqP", [128, DTILES, HPC * DH], DT,
                         kind="ExternalInput").ap()
    wkP = nc.dram_tensor("wkP", [128, DTILES, HPC * DH], DT,
                         kind="ExternalInput").ap()
    wvP = nc.dram_tensor("wvP", [128, DTILES, HPC * DH], DT,
                         kind="ExternalInput").ap()
    woP = nc.dram_tensor("woP", [128, DTILES, OCOLS], DT,
                         kind="ExternalInput").ap()
    cosT = nc.dram_tensor("cosT", [DH, S], DT, kind="ExternalInput").ap()
    srotT = nc.dram_tensor("srotT", [DH, S], DT, kind="ExternalInput").ap()
    masksP = nc.dram_tensor("masksP", [128, 4, SBLK], DT,
                            kind="ExternalInput").ap()
    outT = nc.dram_tensor("outT", [OCOLS, S], F32, kind="ExternalOutput").ap()

    # One AllGather bounce pair per 512-seq block.
    vloc = [nc.dram_tensor(f"vals_loc_{b}", [HPC * DH, SBLK], DT).ap()
            for b in range(NSB)]
    vgath = [nc.dram_tensor(f"vals_gath_{b}", [D, SBLK], DT).ap()
             for b in range(NSB)]
    wup_in = nc.dram_tensor("wup_in", [128, 4], DT).ap()
    wup_out = nc.dram_tensor("wup_out", [512, 4], DT).ap()

    with tc.tile_pool(name="const", bufs=1) as cpool, \
         tc.tile_pool(name="qkv", bufs=1) as qkvpool, \
         tc.tile_pool(name="w", bufs=1) as wpool, \
         tc.tile_pool(name="xs", bufs=2) as xpool, \
         tc.tile_pool(name="rope", bufs=1) as rpool, \
         tc.tile_pool(name="att", bufs=3) as apool, \
         tc.tile_pool(name="vg", bufs=2) as vgpool, \
         tc.tile_pool(name="ob", bufs=2) as obpool:

        cos_s = cpool.tile([128, S], DT, name="cos_s")
        srot_s = cpool.tile([128, S], DT, name="srot_s")
        mask_s = cpool.tile([128, 4, SBLK], DT, name="mask_s")
        # bf16 ones matrix: ones.T @ x sums x over partitions and yields the
        # result replicated across all 128 partitions (DVE cannot broadcast
        # along partitions, so produce the softmax denominator pre-broadcast).
        ones_b = cpool.tile([128, 128], DT, name="ones_b")
        nc.vector.memset(ones_b[:], 1.0)

        qT = qkvpool.tile([128, HPC, S], DT, name="qT")
        kTt = qkvpool.tile([128, HPC, S], DT, name="kTt")
        v_s = qkvpool.tile([128, KT, HPC * DH], DT, name="v_s")

        wq_s = wpool.tile([128, DTILES, HPC * DH], DT, name="wq_s")
        wk_s = wpool.tile([128, DTILES, HPC * DH], DT, name="wk_s")
        wv_s = wpool.tile([128, DTILES, HPC * DH], DT, name="wv_s")
        wo_s = wq_s  # o_proj weights overwrite Wq after the last Q sweep

        def _chunk(dst, src, i, eng=None):
            t0 = i * CSTEP
            (eng or nc.sync).dma_start(dst[:, t0:t0 + CSTEP, :],
                                       src[:, t0:t0 + CSTEP, :])

        x_tiles = {}

        def x_prefetch(sb):
            x_s = xpool.tile([128, DTILES, SBLK], DT, tag="x", name="x_s")
            for i in range(DMA_SPLIT):
                t0 = i * CSTEP
                nc.sync.dma_start(
                    x_s[:, t0:t0 + CSTEP, :],
                    xP[:, sb * DTILES + t0:sb * DTILES + t0 + CSTEP, :])
            x_tiles[sb] = x_s

        # Input loads spread across three DMA queues (sync, scalar, gpsimd):
        # early DMA throughput is descriptor-rate limited per queue, and the
        # Q sweep needs x+Wq as soon as possible after kernel start. The wk
        # triggers go on the gpsimd queue BEFORE the warm-up collective so
        # they are not stuck behind its completion wait.
        x_prefetch(0)
        for i in range(DMA_SPLIT):
            _chunk(wq_s, wqP, i, nc.scalar)
            _chunk(wk_s, wkP, i, nc.gpsimd)
        for i in range(DMA_SPLIT):
            _chunk(wv_s, wvP, i)
        nc.scalar.dma_start(cos_s[:], cosT[:, :])
        nc.scalar.dma_start(srot_s[:], srotT[:, :])
        nc.scalar.dma_start(mask_s[:], masksP[:, :, :])
        # Tiny warm-up AllGather: the first collective of an execution pays
        # ~40us of one-time overhead; absorb it during the projection phase.
        nc.gpsimd.dma_start(wup_in[:, :], ones_b[:, 0:4])
        nc.gpsimd.collective_compute(
            "AllGather", mybir.AluOpType.bypass, replica_groups=RG,
            ins=[wup_in[:, :].opt()], outs=[wup_out[:, :].opt()],
        )

        def rope(ps, dst, s0):
            # RoPE: out = raw*cos + rot(raw)*srot (partition dim = dh)
            cos_b = cos_s[:, s0:s0 + SBLK].unsqueeze(1).broadcast_to(
                [128, HPC, SBLK])
            srot_b = srot_s[:, s0:s0 + SBLK].unsqueeze(1).broadcast_to(
                [128, HPC, SBLK])
            raw = rpool.tile([128, HPC, SBLK], DT, tag="raw", name="raw")
            # split across scalar+vector so the PSUM bank frees fast
            nc.scalar.copy(raw[:, 0:2, :], ps[:, 0:2, :])
            nc.vector.tensor_copy(raw[:, 2:4, :], ps[:, 2:4, :])
            # rotate-half along partitions: engines can't shift partitions,
            # DMA can.
            rot = rpool.tile([128, HPC, SBLK], DT, tag="rot", name="rot")
            nc.scalar.dma_start(rot[0:64], raw[64:128])
            nc.scalar.dma_start(rot[64:128], raw[0:64])
            nc.vector.tensor_mul(rot[:], rot[:], srot_b)
            nc.vector.tensor_mul(raw[:], raw[:], cos_b)
            nc.vector.tensor_add(dst[:, :, s0:s0 + SBLK], raw[:], rot[:])

        # Single PSUM pool shared by every phase: a 4-bank bufs=1 ring
        # (tag "ps") holds the projection accumulators, each attention
        # head's AV+denominator tile, and the o_proj accumulators; score
        # pairs double-buffer through two 2-bank slots (tag "st") as
        # INDEPENDENT tiles - carving them as slices of one big tile (an
        # earlier version) made the Tile tracker serialize the whole
        # attention inner loop at tile granularity.
        with tc.tile_pool(name="pp", bufs=1, space="PSUM") as ppool:

            def proj_block(sb):
                s0 = sb * SBLK
                x_s = x_tiles[sb]
                psq = ppool.tile([128, HPC, SBLK], F32, tag="ps", name="psq")
                for dt_i in range(DTILES):
                    st_ = dt_i == 0
                    sp_ = dt_i == DTILES - 1
                    for h in range(HPC):
                        nc.tensor.matmul(
                            psq[:, h, :],
                            lhsT=wq_s[:, dt_i, h * DH:(h + 1) * DH],
                            rhs=x_s[:, dt_i, :],
                            start=st_, stop=sp_,
                        )
                rope(psq, qT, s0)
                if sb == NSB - 1:
                    # overwrite Wq (its last consumer was this block's Q
                    # sweep) with the o_proj weights; lands long before
                    # o_proj starts.
                    for i in range(DMA_SPLIT):
                        _chunk(wq_s, woP, i)
                psk = ppool.tile([128, HPC, SBLK], F32, tag="ps", name="psk")
                for dt_i in range(DTILES):
                    st_ = dt_i == 0
                    sp_ = dt_i == DTILES - 1
                    for h in range(HPC):
                        nc.tensor.matmul(
                            psk[:, h, :],
                            lhsT=wk_s[:, dt_i, h * DH:(h + 1) * DH],
                            rhs=x_s[:, dt_i, :],
                            start=st_, stop=sp_,
                        )
                rope(psk, kTt, s0)
                # V sweep: x^T tiles stationary, W_v moving
                psv = ppool.tile([128, HPC, SBLK], F32, tag="ps", name="psv")
                for dt_i in range(DTILES):
                    st_ = dt_i == 0
                    sp_ = dt_i == DTILES - 1
                    for st in range(4):  # seq sub-tiles of this block
                        nc.tensor.matmul(
                            psv[:, st, :],
                            lhsT=x_s[:, dt_i, st * 128:(st + 1) * 128],
                            rhs=wv_s[:, dt_i, :],
                            start=st_, stop=sp_,
                        )
                for st in range(4):
                    if st < 2:
                        nc.scalar.copy(v_s[:, sb * 4 + st, :], psv[:, st, :])
                    else:
                        nc.vector.tensor_copy(v_s[:, sb * 4 + st, :],
                                              psv[:, st, :])

            pending_fin = [None]

            def attn_head(sqb, h):
                sq0 = sqb * SBLK
                nkt = 4 * (sqb + 1)
                npair = nkt // 2
                # AV accumulator and softmax denominator live in one 4-bank
                # ring tile (banks 2-3 unused - the ring slot is 4 banks).
                avdn = ppool.tile([128, HPC, SBLK], F32, tag="ps",
                                  name="avdn")
                ps_av = avdn[:, 0, :]
                qsum = apool.tile([128, SBLK], DT, tag="qsum", name="qsum",
                                  bufs=2)

                def emit_av(p, st_e):
                    for i in range(2):
                        kt = 2 * p + i
                        nc.tensor.matmul(
                            ps_av,
                            lhsT=v_s[:, kt, h * DH:(h + 1) * DH],
                            rhs=st_e[:, i, :],
                            start=(kt == 0), stop=(kt == nkt - 1),
                        )

                # Software-pipelined over score pairs (see module docstring).
                prev = None
                for p in range(npair):
                    ps_st = ppool.tile([128, 2, SBLK], F32, tag="st",
                                       name="ps_st", bufs=2)
                    for i in range(2):
                        kt = 2 * p + i
                        nc.tensor.matmul(
                            ps_st[:, i, :],
                            lhsT=kTt[:, h, kt * 128:(kt + 1) * 128],
                            rhs=qT[:, h, sq0:sq0 + SBLK],
                            start=True, stop=True,
                        )
                    if p == 1 and pending_fin[0] is not None:
                        # previous head's denominator matmul + rescale,
                        # deferred here so the PE does not wait on that
                        # head's trailing exp -> DVE accumulation chain.
                        pending_fin[0]()
                        pending_fin[0] = None
                    st_e = apool.tile([128, 2, SBLK], DT, tag="ste",
                                      name="st_e", bufs=3)
                    nc.scalar.activation(st_e[:], ps_st[:], AF.Exp,
                                         scale=SCALE)
                    pm = 2 * p - (nkt - 4)
                    if pm >= 0:  # diagonal pair: causal 0/1 mask
                        nc.vector.tensor_mul(st_e[:], st_e[:],
                                             mask_s[:, pm:pm + 2, :])
                    # softmax denominator: accumulate exp tiles on the DVE
                    # (bf16), one ones-matmul at the end reduces over
                    # partitions.
                    if p == 0:
                        nc.vector.tensor_add(qsum[:], st_e[:, 0, :],
                                             st_e[:, 1, :])
                    else:
                        nc.vector.tensor_add(qsum[:], qsum[:], st_e[:, 0, :])
                        nc.vector.tensor_add(qsum[:], qsum[:], st_e[:, 1, :])
                    if prev is not None:
                        emit_av(p - 1, prev)
                    prev = st_e
                emit_av(npair - 1, prev)

                def fin():
                    ps_den = avdn[:, 1, :]
                    nc.tensor.matmul(ps_den, lhsT=ones_b[:], rhs=qsum[:],
                                     start=True, stop=True)
                    rden = apool.tile([128, SBLK], F32, tag="rden",
                                      name="rden", bufs=2)
                    nc.vector.reciprocal_approx_fast(rden[:], ps_den)
                    vout = apool.tile([128, SBLK], DT, tag="vout",
                                      name="vout", bufs=4)
                    nc.vector.tensor_mul(vout[:], ps_av, rden[:])
                    nc.sync.dma_start(
                        vloc[sqb][h * DH:(h + 1) * DH, :], vout[:])
                pending_fin[0] = fin

            def attn_block(sqb):
                for h in range(HPC):
                    attn_head(sqb, h)
                # flush the last head's finisher before the AllGather is
                # emitted (its vloc write must precede the collective in
                # program order for dependency tracking).
                pending_fin[0]()
                pending_fin[0] = None

            vg_tiles = []

            def ag_block(b):
                nc.gpsimd.collective_compute(
                    "AllGather", mybir.AluOpType.bypass, replica_groups=RG,
                    ins=[vloc[b][:, :].opt()], outs=[vgath[b][:, :].opt()],
                )
                # Load the gathered vals to SBUF right away: the gpsimd
                # engine just waited for this AllGather's completion anyway,
                # so the load fires immediately.
                vg = vgpool.tile([128, DTILES, SBLK], DT, tag="vg", name="vg")
                for i_ in range(2):
                    t0 = i_ * (DTILES // 2)
                    nc.gpsimd.dma_start(
                        vg[:, t0:t0 + DTILES // 2, :],
                        vgath[b][t0 * 128:(t0 + DTILES // 2) * 128, :]
                        .rearrange("(t p) s -> p t s", p=128))
                vg_tiles.append(vg)

            def oproj_block(b):
                s0 = b * SBLK
                vg = vg_tiles[b]
                for ct in range(OCOLS // 128):
                    # one bank of a 2-bank "st" ring slot - reusing the
                    # score-pair ring for double buffering
                    ps_o = ppool.tile([128, 2, SBLK], F32, tag="st",
                                      name="ps_o", bufs=2)
                    for dt_i in range(DTILES):
                        nc.tensor.matmul(
                            ps_o[:, 0, :],
                            lhsT=wo_s[:, dt_i, ct * 128:(ct + 1) * 128],
                            rhs=vg[:, dt_i, :],
                            start=(dt_i == 0), stop=(dt_i == DTILES - 1),
                        )
                    ob = obpool.tile([128, SBLK], F32, tag="ob", name="ob")
                    nc.scalar.copy(ob[:], ps_o[:, 0, :])
                    nc.scalar.dma_start(
                        outT[ct * 128:(ct + 1) * 128, s0:s0 + SBLK], ob[:])

            # ---- main schedule: proj(b) -> attn(b) -> AG(b); o_proj tail
            for sb in range(NSB):
                if sb + 1 < NSB:
                    x_prefetch(sb + 1)
                proj_block(sb)
                attn_block(sb)
                ag_block(sb)
            for c in range(NSB):
                with tc.tile_wait_until(0.45 + 0.01 * c):
                    oproj_block(c)


_NC_CACHE = None


def build_program():
    global _NC_CACHE
    if _NC_CACHE is not None:
        return _NC_CACHE
    nc = bacc.Bacc("TRN2", target_bir_lowering=False, debug=False,
                   enable_asserts=False, num_devices=NCORES)
    with tile.TileContext(nc) as tc:
        _emit(tc)
    nc.compile()
    _NC_CACHE = nc
    return nc


def _pack_pmaj(a2d):
    """[T*128, C] row-major -> [128, T, C] with per-partition-contiguous
    lines (partition p holds rows p, 128+p, ... consecutively)."""
    t = a2d.shape[0] // 128
    return np.ascontiguousarray(
        a2d.reshape(t, 128, a2d.shape[1]).transpose(1, 0, 2)).astype(NPDT)


def _prep_inputs(x, cos, sin, Wq, Wk, Wv, Wo):
    """Build the 8 per-core input maps (host-side sharding only)."""
    x = np.asarray(x, dtype=np.float32)
    cos = np.asarray(cos, dtype=np.float32)
    sin = np.asarray(sin, dtype=np.float32)
    Wq = np.asarray(Wq, dtype=np.float32)
    Wk = np.asarray(Wk, dtype=np.float32)
    Wv = np.asarray(Wv, dtype=np.float32)
    Wo = np.asarray(Wo, dtype=np.float32)

    cosT = np.ascontiguousarray(cos.T).astype(NPDT)             # [128, S]
    sinT = np.ascontiguousarray(sin.T)
    srotT = np.concatenate([-sinT[:64], sinT[64:]], axis=0).astype(NPDT)

    iota = np.arange(SBLK)[None, :]
    rows = np.arange(128)[:, None]
    masks = np.stack(
        [(128 * p + rows <= iota) for p in range(4)]).astype(NPDT)  # [4,128,512]
    masksP = np.ascontiguousarray(masks.transpose(1, 0, 2))         # [128,4,512]

    # x packed [128, NSB*DTILES, SBLK]: element [p, sb*16+t, s] =
    # x[g][sb*512+s, t*128+p], per-(p,sb) line 16KB contiguous.
    xPg = [np.ascontiguousarray(
        x[g].T.reshape(DTILES, 128, NSB, SBLK).transpose(1, 2, 0, 3)
        .reshape(128, NSB * DTILES, SBLK)).astype(NPDT) for g in range(B)]

    in_maps = []
    for c in range(NCORES):
        g, r = c // TP, c % TP
        hs = slice(r * HPC * DH, (r + 1) * HPC * DH)
        in_maps.append({
            "xP": xPg[g],
            "wqP": _pack_pmaj(Wq[hs].T),
            "wkP": _pack_pmaj(Wk[hs].T),
            "wvP": _pack_pmaj(Wv[hs].T),
            "woP": _pack_pmaj(Wo[r * OCOLS:(r + 1) * OCOLS].T),
            "cosT": cosT,
            "srotT": srotT,
            "masksP": masksP,
        })
    return in_maps


def run(inputs, trace=False, trace_cores=None):
    nc = build_program()
    in_maps = _prep_inputs(**inputs)
    res = run_bass_kernel_spmd(
        nc, in_maps, core_ids=list(range(NCORES)),
        trace=trace, trace_cores=trace_cores,
    )
    out = np.empty((B, S, D), dtype=np.float32)
    for c in range(NCORES):
        g, r = c // TP, c % TP
        out[g, :, r * OCOLS:(r + 1) * OCOLS] = res.results[c]["outT"].T
    return out, res


def kernel(**inputs):
    out, _ = run(inputs)
    return out


# revision 20
# speedup vs baseline: 1.0285x; 1.0285x over previous
"""Llama attention layer (B=2, S=2048, D=2048, H=16, DH=128) on 8 TRN2 NeuronCores.

Sharding: 2-way data parallel over batch x 4-way tensor parallel over heads.
Core c: batch g = c // 4, heads 4r..4r+3 where r = c % 4.
Projections are column-parallel (each core computes Q/K/V for its 4 heads),
attention is fully local per (batch, head), then the per-head attention
outputs (kept transposed, [dim, seq]) are AllGather'd within each 4-core
batch group, and o_proj is column-parallel: core c computes output columns
r*512..(r+1)*512 of its batch. Host concatenates - no host-side compute.

Schedule: attention blocks run in ascending order; their outputs are
gathered with just TWO AllGathers (blocks {0,1} and {2,3}) because
collectives serialize on the issuing engine - with four gathers the chain
(25-45us per op, straggler-bound) finished ~40us after the last o_proj
consumer needed it. AG{01} fires at attn1-end and AG{23} at attn3-end, both
input-bound; o_proj runs at the end of the PE stream consuming blocks in
order 0..3, so only o_proj2 can briefly wait. All inputs are packed
partition-major on the host so DMA lines are 4KB+, spread over three DMA
queues (early DMA is descriptor-rate limited per queue).

All matmul operands are bf16 (fp32 accumulation in PSUM); softmax runs
without max-subtraction (scores are O(6), exp is safe in fp32); the
denominator is accumulated on the vector engine in bf16 and reduced over
partitions with a single ones-matmul per (head, seq-block), deferred into
the next head's stream so the PE never waits on the trailing exp chain. The
attention inner loop is software-pipelined: pair p+1's QK matmuls are
emitted before pair p's AV matmuls, hiding the scalar-engine exp latency.
"""

import os
import sys

for _p in ("/opt/trn_rl_repo", "/root/.axon_site/_ro/trn_rl_repo"):
    if os.path.isdir(_p) and _p not in sys.path:
        sys.path.append(_p)

import numpy as np
import ml_dtypes

import concourse.bass as bass
import concourse.tile as tile
import concourse.mybir as mybir
from concourse import bacc
from concourse.bass_utils import run_bass_kernel_spmd

F32 = mybir.dt.float32
BF16 = mybir.dt.bfloat16
AF = mybir.ActivationFunctionType

B, S, D, H, DH = 2, 2048, 2048, 16, 128
NCORES = 8
TP = 4                 # cores per batch group
HPC = H // TP          # heads per core = 4
SBLK = 512             # seq block (matmul moving size)
NSB = S // SBLK        # 4
DTILES = D // 128      # 16 contraction tiles
KT = S // 128          # 16 key tiles
OCOLS = D // TP        # 512 output columns per core
SCALE = 1.0 / float(np.sqrt(DH))
DMA_SPLIT = 4          # split big input DMAs so compute starts early
CSTEP = DTILES // DMA_SPLIT
RG = [[0, 1, 2, 3], [4, 5, 6, 7]]

DT = BF16              # matmul operand dtype
NPDT = ml_dtypes.bfloat16


def _emit(tc):
    nc = tc.nc
    # Inputs packed partition-major on host: per partition line contiguous.
    xP = nc.dram_tensor("xP", [128, NSB * DTILES, SBLK], DT,
                        kind="ExternalInput").ap()
    wqP = nc.dram_tensor("wqP", [128, DTILES, HPC * DH], DT,
                         kind="ExternalInput").ap()
    wkP = nc.dram_tensor("wkP", [128, DTILES, HPC * DH], DT,
                         kind="ExternalInput").ap()
    wvP = nc.dram_tensor("wvP", [128, DTILES, HPC * DH], DT,
                         kind="ExternalInput").ap()
    woP = nc.dram_tensor("woP", [128, DTILES, OCOLS], DT,
                         kind="ExternalInput").ap()
    cosT = nc.dram_tensor("cosT", [DH, S], DT, kind="ExternalInput").ap()
    srotT = nc.dram_tensor("srotT", [DH, S], DT, kind="ExternalInput").ap()
    masksP = nc.dram_tensor("masksP", [128, 4, SBLK], DT,
                            kind="ExternalInput").ap()
    outT = nc.dram_tensor("outT", [OCOLS, S], F32, kind="ExternalOutput").ap()

    # One AllGather bounce pair per 512-seq block.
    vloc = [nc.dram_tensor(f"vals_loc_{b}", [HPC * DH, SBLK], DT).ap()
            for b in range(NSB)]
    vgath = [nc.dram_tensor(f"vals_gath_{b}", [D, SBLK], DT).ap()
             for b in range(NSB)]
    wup_in = nc.dram_tensor("wup_in", [128, 4], DT).ap()
    wup_out = nc.dram_tensor("wup_out", [512, 4], DT).ap()

    with tc.tile_pool(name="const", bufs=1) as cpool, \
         tc.tile_pool(name="qkv", bufs=1) as qkvpool, \
         tc.tile_pool(name="w", bufs=1) as wpool, \
         tc.tile_pool(name="xs", bufs=2) as xpool, \
         tc.tile_pool(name="rope", bufs=1) as rpool, \
         tc.tile_pool(name="att", bufs=3) as apool, \
         tc.tile_pool(name="vg", bufs=2) as vgpool, \
         tc.tile_pool(name="ob", bufs=2) as obpool:

        cos_s = cpool.tile([128, S], DT, name="cos_s")
        srot_s = cpool.tile([128, S], DT, name="srot_s")
        mask_s = cpool.tile([128, 4, SBLK], DT, name="mask_s")
        # bf16 ones matrix: ones.T @ x sums x over partitions and yields the
        # result replicated across all 128 partitions (DVE cannot broadcast
        # along partitions, so produce the softmax denominator pre-broadcast).
        ones_b = cpool.tile([128, 128], DT, name="ones_b")
        nc.vector.memset(ones_b[:], 1.0)

        qT = qkvpool.tile([128, HPC, S], DT, name="qT")
        kTt = qkvpool.tile([128, HPC, S], DT, name="kTt")
        v_s = qkvpool.tile([128, KT, HPC * DH], DT, name="v_s")

        wq_s = wpool.tile([128, DTILES, HPC * DH], DT, name="wq_s")
        wk_s = wpool.tile([128, DTILES, HPC * DH], DT, name="wk_s")
        wv_s = wpool.tile([128, DTILES, HPC * DH], DT, name="wv_s")
        wo_s = wq_s  # o_proj weights overwrite Wq after the last Q sweep

        def _chunk(dst, src, i, eng=None):
            t0 = i * CSTEP
            (eng or nc.sync).dma_start(dst[:, t0:t0 + CSTEP, :],
                                       src[:, t0:t0 + CSTEP, :])

        x_tiles = {}

        def x_prefetch(sb):
            x_s = xpool.tile([128, DTILES, SBLK], DT, tag="x", name="x_s")
            for i in range(DMA_SPLIT):
                t0 = i * CSTEP
                nc.sync.dma_start(
                    x_s[:, t0:t0 + CSTEP, :],
                    xP[:, sb * DTILES + t0:sb * DTILES + t0 + CSTEP, :])
            x_tiles[sb] = x_s

        # Input loads spread across three DMA queues (sync, scalar, gpsimd):
        # early DMA throughput is descriptor-rate limited per queue, and the
        # Q sweep needs x+Wq as soon as possible after kernel start. The wk
        # triggers go on the gpsimd queue BEFORE the warm-up collective so
        # they are not stuck behind its completion wait.
        x_prefetch(0)
        for i in range(DMA_SPLIT):
            _chunk(wq_s, wqP, i, nc.scalar)
            _chunk(wk_s, wkP, i, nc.gpsimd)
        for i in range(DMA_SPLIT):
            _chunk(wv_s, wvP, i)
        nc.scalar.dma_start(cos_s[:], cosT[:, :])
        nc.scalar.dma_start(srot_s[:], srotT[:, :])
        nc.scalar.dma_start(mask_s[:], masksP[:, :, :])
        # Tiny warm-up AllGather: the first collective of an execution pays
        # ~40us of one-time overhead; absorb it during the projection phase.
        nc.gpsimd.dma_start(wup_in[:, :], ones_b[:, 0:4])
        nc.gpsimd.collective_compute(
            "AllGather", mybir.AluOpType.bypass, replica_groups=RG,
            ins=[wup_in[:, :].opt()], outs=[wup_out[:, :].opt()],
        )

        def rope(ps, dst, s0):
            # RoPE: out = raw*cos + rot(raw)*srot (partition dim = dh)
            cos_b = cos_s[:, s0:s0 + SBLK].unsqueeze(1).broadcast_to(
                [128, HPC, SBLK])
            srot_b = srot_s[:, s0:s0 + SBLK].unsqueeze(1).broadcast_to(
                [128, HPC, SBLK])
            raw = rpool.tile([128, HPC, SBLK], DT, tag="raw", name="raw")
            # split across scalar+vector so the PSUM bank frees fast
            nc.scalar.copy(raw[:, 0:2, :], ps[:, 0:2, :])
            nc.vector.tensor_copy(raw[:, 2:4, :], ps[:, 2:4, :])
            # rotate-half along partitions: engines can't shift partitions,
            # DMA can.
            rot = rpool.tile([128, HPC, SBLK], DT, tag="rot", name="rot")
            nc.scalar.dma_start(rot[0:64], raw[64:128])
            nc.scalar.dma_start(rot[64:128], raw[0:64])
            nc.vector.tensor_mul(rot[:], rot[:], srot_b)
            nc.vector.tensor_mul(raw[:], raw[:], cos_b)
            nc.vector.tensor_add(dst[:, :, s0:s0 + SBLK], raw[:], rot[:])

        # ---------------- projections + RoPE ----------------
        with tc.tile_pool(name="pp", bufs=2, space="PSUM") as ppool:
            for sb in range(NSB):
                s0 = sb * SBLK
                if sb + 1 < NSB:
                    x_prefetch(sb + 1)
                x_s = x_tiles[sb]
                psq = ppool.tile([128, HPC, SBLK], F32, tag="ps", name="psq")
                for dt_i in range(DTILES):
                    st_ = dt_i == 0
                    sp_ = dt_i == DTILES - 1
                    for h in range(HPC):
                        nc.tensor.matmul(
                            psq[:, h, :],
                            lhsT=wq_s[:, dt_i, h * DH:(h + 1) * DH],
                            rhs=x_s[:, dt_i, :],
                            start=st_, stop=sp_,
                        )
                rope(psq, qT, s0)
                if sb == NSB - 1:
                    # overwrite Wq (its last consumer was this block's Q
                    # sweep) with the o_proj weights; lands long before
                    # o_proj starts.
                    for i in range(DMA_SPLIT):
                        _chunk(wq_s, woP, i)
                psk = ppool.tile([128, HPC, SBLK], F32, tag="ps", name="psk")
                for dt_i in range(DTILES):
                    st_ = dt_i == 0
                    sp_ = dt_i == DTILES - 1
                    for h in range(HPC):
                        nc.tensor.matmul(
                            psk[:, h, :],
                            lhsT=wk_s[:, dt_i, h * DH:(h + 1) * DH],
                            rhs=x_s[:, dt_i, :],
                            start=st_, stop=sp_,
                        )
                rope(psk, kTt, s0)
                # V sweep: x^T tiles stationary, W_v moving
                psv = ppool.tile([128, HPC, SBLK], F32, tag="ps", name="psv")
                for dt_i in range(DTILES):
                    st_ = dt_i == 0
                    sp_ = dt_i == DTILES - 1
                    for st in range(4):  # seq sub-tiles of this block
                        nc.tensor.matmul(
                            psv[:, st, :],
                            lhsT=x_s[:, dt_i, st * 128:(st + 1) * 128],
                            rhs=wv_s[:, dt_i, :],
                            start=st_, stop=sp_,
                        )
                for st in range(4):
                    if st < 2:
                        nc.scalar.copy(v_s[:, sb * 4 + st, :], psv[:, st, :])
                    else:
                        nc.vector.tensor_copy(v_s[:, sb * 4 + st, :],
                                              psv[:, st, :])

        # ---------------- attention + AG, then o_proj ----------------
        with tc.tile_pool(name="aps", bufs=2, space="PSUM") as apsum:
            pending_fin = [None]

            def attn_head(sqb, h):
                sq0 = sqb * SBLK
                nkt = 4 * (sqb + 1)
                npair = nkt // 2
                ps_av = apsum.tile([128, SBLK], F32, tag="av", name="ps_av")
                qsum = apool.tile([128, SBLK], DT, tag="qsum", name="qsum",
                                  bufs=2)

                def emit_av(p, st_e):
                    for i in range(2):
                        kt = 2 * p + i
                        nc.tensor.matmul(
                            ps_av[:],
                            lhsT=v_s[:, kt, h * DH:(h + 1) * DH],
                            rhs=st_e[:, i, :],
                            start=(kt == 0), stop=(kt == nkt - 1),
                        )

                # Software-pipelined over score pairs (see module docstring).
                prev = None
                for p in range(npair):
                    ps_st = apsum.tile([128, 2, SBLK], F32, tag="st",
                                       name="ps_st", bufs=2)
                    for i in range(2):
                        kt = 2 * p + i
                        nc.tensor.matmul(
                            ps_st[:, i, :],
                            lhsT=kTt[:, h, kt * 128:(kt + 1) * 128],
                            rhs=qT[:, h, sq0:sq0 + SBLK],
                            start=True, stop=True,
                        )
                    if p == 1 and pending_fin[0] is not None:
                        # previous head's denominator matmul + rescale,
                        # deferred here so the PE does not wait on that
                        # head's trailing exp -> DVE accumulation chain.
                        pending_fin[0]()
                        pending_fin[0] = None
                    st_e = apool.tile([128, 2, SBLK], DT, tag="ste",
                                      name="st_e", bufs=3)
                    nc.scalar.activation(st_e[:], ps_st[:], AF.Exp,
                                         scale=SCALE)
                    pm = 2 * p - (nkt - 4)
                    if pm >= 0:  # diagonal pair: causal 0/1 mask
                        nc.vector.tensor_mul(st_e[:], st_e[:],
                                             mask_s[:, pm:pm + 2, :])
                    # softmax denominator: accumulate exp tiles on the DVE
                    # (bf16), one ones-matmul at the end reduces over
                    # partitions.
                    if p == 0:
                        nc.vector.tensor_add(qsum[:], st_e[:, 0, :],
                                             st_e[:, 1, :])
                    else:
                        nc.vector.tensor_add(qsum[:], qsum[:], st_e[:, 0, :])
                        nc.vector.tensor_add(qsum[:], qsum[:], st_e[:, 1, :])
                    if prev is not None:
                        emit_av(p - 1, prev)
                    prev = st_e
                emit_av(npair - 1, prev)

                def fin():
                    ps_den = apsum.tile([128, SBLK], F32, tag="dn",
                                        name="ps_den", bufs=2)
                    nc.tensor.matmul(ps_den[:], lhsT=ones_b[:], rhs=qsum[:],
                                     start=True, stop=True)
                    rden = apool.tile([128, SBLK], F32, tag="rden",
                                      name="rden", bufs=2)
                    nc.vector.reciprocal_approx_fast(rden[:], ps_den[:])
                    vout = apool.tile([128, SBLK], DT, tag="vout",
                                      name="vout", bufs=4)
                    nc.vector.tensor_mul(vout[:], ps_av[:], rden[:])
                    nc.sync.dma_start(
                        vloc[sqb][h * DH:(h + 1) * DH, :], vout[:])
                pending_fin[0] = fin

            def attn_block(sqb):
                for h in range(HPC):
                    attn_head(sqb, h)
                # flush the last head's finisher before a following
                # AllGather is emitted (its vloc write must precede the
                # collective in program order for dependency tracking).
                pending_fin[0]()
                pending_fin[0] = None

            vg_tiles = []

            def ag_block(b):
                # Collective only - interleaving the gathered-vals loads on
                # the gpsimd queue (an earlier version) delayed the next
                # doorbell and made every AllGather 1.5-2x slower.
                nc.gpsimd.collective_compute(
                    "AllGather", mybir.AluOpType.bypass, replica_groups=RG,
                    ins=[vloc[b][:, :].opt()], outs=[vgath[b][:, :].opt()],
                )

            def vg_load(b):
                # Post-attention, the sync engine/queue is idle: its blocking
                # wait on AllGather b's completion delays nothing else.
                vg = vgpool.tile([128, DTILES, SBLK], DT, tag="vg", name="vg")
                for i_ in range(2):
                    t0 = i_ * (DTILES // 2)
                    nc.sync.dma_start(
                        vg[:, t0:t0 + DTILES // 2, :],
                        vgath[b][t0 * 128:(t0 + DTILES // 2) * 128, :]
                        .rearrange("(t p) s -> p t s", p=128))
                vg_tiles.append(vg)

            def oproj_block(b):
                s0 = b * SBLK
                vg = vg_tiles[b]
                for ct in range(OCOLS // 128):
                    ps_o = apsum.tile([128, SBLK], F32, tag="dn", name="ps_o",
                                      bufs=2)
                    for dt_i in range(DTILES):
                        nc.tensor.matmul(
                            ps_o[:],
                            lhsT=wo_s[:, dt_i, ct * 128:(ct + 1) * 128],
                            rhs=vg[:, dt_i, :],
                            start=(dt_i == 0), stop=(dt_i == DTILES - 1),
                        )
                    ob = obpool.tile([128, SBLK], F32, tag="ob", name="ob")
                    nc.scalar.copy(ob[:], ps_o[:])
                    nc.scalar.dma_start(
                        outT[ct * 128:(ct + 1) * 128, s0:s0 + SBLK], ob[:])

            for sqb in range(NSB):
                attn_block(sqb)
                ag_block(sqb)
            for c in range(NSB):
                vg_load(c)
            for c in range(NSB):
                with tc.tile_wait_until(0.45 + 0.01 * c):
                    oproj_block(c)


_NC_CACHE = None


def build_program():
    global _NC_CACHE
    if _NC_CACHE is not None:
        return _NC_CACHE
    nc = bacc.Bacc("TRN2", target_bir_lowering=False, debug=False,
                   enable_asserts=False, num_devices=NCORES)
    with tile.TileContext(nc) as tc:
        _emit(tc)
    nc.compile()
    _NC_CACHE = nc
    return nc


def _pack_pmaj(a2d):
    """[T*128, C] row-major -> [128, T, C] with per-partition-contiguous
    lines (partition p holds rows p, 128+p, ... consecutively)."""
    t = a2d.shape[0] // 128
    return np.ascontiguousarray(
        a2d.reshape(t, 128, a2d.shape[1]).transpose(1, 0, 2)).astype(NPDT)


def _prep_inputs(x, cos, sin, Wq, Wk, Wv, Wo):
    """Build the 8 per-core input maps (host-side sharding only)."""
    x = np.asarray(x, dtype=np.float32)
    cos = np.asarray(cos, dtype=np.float32)
    sin = np.asarray(sin, dtype=np.float32)
    Wq = np.asarray(Wq, dtype=np.float32)
    Wk = np.asarray(Wk, dtype=np.float32)
    Wv = np.asarray(Wv, dtype=np.float32)
    Wo = np.asarray(Wo, dtype=np.float32)

    cosT = np.ascontiguousarray(cos.T).astype(NPDT)             # [128, S]
    sinT = np.ascontiguousarray(sin.T)
    srotT = np.concatenate([-sinT[:64], sinT[64:]], axis=0).astype(NPDT)

    iota = np.arange(SBLK)[None, :]
    rows = np.arange(128)[:, None]
    masks = np.stack(
        [(128 * p + rows <= iota) for p in range(4)]).astype(NPDT)  # [4,128,512]
    masksP = np.ascontiguousarray(masks.transpose(1, 0, 2))         # [128,4,512]

    # x packed [128, NSB*DTILES, SBLK]: element [p, sb*16+t, s] =
    # x[g][sb*512+s, t*128+p], per-(p,sb) line 16KB contiguous.
    xPg = [np.ascontiguousarray(
        x[g].T.reshape(DTILES, 128, NSB, SBLK).transpose(1, 2, 0, 3)
        .reshape(128, NSB * DTILES, SBLK)).astype(NPDT) for g in range(B)]

    in_maps = []
    for c in range(NCORES):
        g, r = c // TP, c % TP
        hs = slice(r * HPC * DH, (r + 1) * HPC * DH)
        in_maps.append({
            "xP": xPg[g],
            "wqP": _pack_pmaj(Wq[hs].T),
            "wkP": _pack_pmaj(Wk[hs].T),
            "wvP": _pack_pmaj(Wv[hs].T),
            "woP": _pack_pmaj(Wo[r * OCOLS:(r + 1) * OCOLS].T),
            "cosT": cosT,
            "srotT": srotT,
            "masksP": masksP,
        })
    return in_maps


def run(inputs, trace=False, trace_cores=None):
    nc = build_program()
    in_maps = _prep_inputs(**inputs)
    res = run_bass_kernel_spmd(
        nc, in_maps, core_ids=list(range(NCORES)),
        trace=trace, trace_cores=trace_cores,
    )
    out = np.empty((B, S, D), dtype=np.float32)
    for c in range(NCORES):
        g, r = c // TP, c % TP
        out[g, :, r * OCOLS:(r + 1) * OCOLS] = res.results[c]["outT"].T
    return out, res


def kernel(**inputs):
    out, _ = run(inputs)
    return out


# revision 21
# speedup vs baseline: 1.0767x; 1.0468x over previous
"""Llama attention layer (B=2, S=2048, D=2048, H=16, DH=128) on 8 TRN2 NeuronCores.

Sharding: 2-way data parallel over batch x 4-way tensor parallel over heads.
Core c: batch g = c // 4, heads 4r..4r+3 where r = c % 4.
Projections are column-parallel (each core computes Q/K/V for its 4 heads),
attention is fully local per (batch, head), then the per-head attention
outputs (kept transposed, [dim, seq]) are AllGather'd within each 4-core
batch group, and o_proj is column-parallel: core c computes output columns
r*512..(r+1)*512 of its batch. Host concatenates - no host-side compute.

Schedule: attention blocks run in ascending order; their outputs are
gathered with just TWO AllGathers (blocks {0,1} and {2,3}) because
collectives serialize on the issuing engine - with four gathers the chain
(25-45us per op, straggler-bound) finished ~40us after the last o_proj
consumer needed it. AG{01} fires at attn1-end and AG{23} at attn3-end, both
input-bound; o_proj runs at the end of the PE stream consuming blocks in
order 0..3, so only o_proj2 can briefly wait. All inputs are packed
partition-major on the host so DMA lines are 4KB+, spread over three DMA
queues (early DMA is descriptor-rate limited per queue).

All matmul operands are bf16 (fp32 accumulation in PSUM); softmax runs
without max-subtraction (scores are O(6), exp is safe in fp32); the
denominator is accumulated on the vector engine in bf16 and reduced over
partitions with a single ones-matmul per (head, seq-block), deferred into
the next head's stream so the PE never waits on the trailing exp chain. The
attention inner loop is software-pipelined: pair p+1's QK matmuls are
emitted before pair p's AV matmuls, hiding the scalar-engine exp latency.
"""

import os
import sys

for _p in ("/opt/trn_rl_repo", "/root/.axon_site/_ro/trn_rl_repo"):
    if os.path.isdir(_p) and _p not in sys.path:
        sys.path.append(_p)

import numpy as np
import ml_dtypes

import concourse.bass as bass
import concourse.tile as tile
import concourse.mybir as mybir
from concourse import bacc
from concourse.bass_utils import run_bass_kernel_spmd

F32 = mybir.dt.float32
BF16 = mybir.dt.bfloat16
AF = mybir.ActivationFunctionType

B, S, D, H, DH = 2, 2048, 2048, 16, 128
NCORES = 8
TP = 4                 # cores per batch group
HPC = H // TP          # heads per core = 4
SBLK = 512             # seq block (matmul moving size)
NSB = S // SBLK        # 4
DTILES = D // 128      # 16 contraction tiles
KT = S // 128          # 16 key tiles
OCOLS = D // TP        # 512 output columns per core
SCALE = 1.0 / float(np.sqrt(DH))
DMA_SPLIT = 4          # split big input DMAs so compute starts early
CSTEP = DTILES // DMA_SPLIT
RG = [[0, 1, 2, 3], [4, 5, 6, 7]]

DT = BF16              # matmul operand dtype
NPDT = ml_dtypes.bfloat16


def _emit(tc):
    nc = tc.nc
    # Inputs packed partition-major on host: per partition line contiguous.
    xP = nc.dram_tensor("xP", [128, NSB * DTILES, SBLK], DT,
                        kind="ExternalInput").ap()
    wqP = nc.dram_tensor("wqP", [128, DTILES, HPC * DH], DT,
                         kind="ExternalInput").ap()
    wkP = nc.dram_tensor("wkP", [128, DTILES, HPC * DH], DT,
                         kind="ExternalInput").ap()
    wvP = nc.dram_tensor("wvP", [128, DTILES, HPC * DH], DT,
                         kind="ExternalInput").ap()
    woP = nc.dram_tensor("woP", [128, DTILES, OCOLS], DT,
                         kind="ExternalInput").ap()
    cosT = nc.dram_tensor("cosT", [DH, S], DT, kind="ExternalInput").ap()
    srotT = nc.dram_tensor("srotT", [DH, S], DT, kind="ExternalInput").ap()
    masksP = nc.dram_tensor("masksP", [128, 4, SBLK], DT,
                            kind="ExternalInput").ap()
    outT = nc.dram_tensor("outT", [OCOLS, S], F32, kind="ExternalOutput").ap()

    # One AllGather bounce pair per 512-seq block.
    vloc = [nc.dram_tensor(f"vals_loc_{b}", [HPC * DH, SBLK], DT).ap()
            for b in range(NSB)]
    vgath = [nc.dram_tensor(f"vals_gath_{b}", [D, SBLK], DT).ap()
             for b in range(NSB)]
    wup_in = nc.dram_tensor("wup_in", [128, 4], DT).ap()
    wup_out = nc.dram_tensor("wup_out", [512, 4], DT).ap()

    with tc.tile_pool(name="const", bufs=1) as cpool, \
         tc.tile_pool(name="qkv", bufs=1) as qkvpool, \
         tc.tile_pool(name="w", bufs=1) as wpool, \
         tc.tile_pool(name="xs", bufs=2) as xpool, \
         tc.tile_pool(name="rope", bufs=1) as rpool, \
         tc.tile_pool(name="att", bufs=3) as apool, \
         tc.tile_pool(name="vg", bufs=2) as vgpool, \
         tc.tile_pool(name="ob", bufs=2) as obpool:

        cos_s = cpool.tile([128, S], DT, name="cos_s")
        srot_s = cpool.tile([128, S], DT, name="srot_s")
        mask_s = cpool.tile([128, 4, SBLK], DT, name="mask_s")
        # bf16 ones matrix: ones.T @ x sums x over partitions and yields the
        # result replicated across all 128 partitions (DVE cannot broadcast
        # along partitions, so produce the softmax denominator pre-broadcast).
        ones_b = cpool.tile([128, 128], DT, name="ones_b")
        nc.vector.memset(ones_b[:], 1.0)

        qT = qkvpool.tile([128, HPC, S], DT, name="qT")
        kTt = qkvpool.tile([128, HPC, S], DT, name="kTt")
        v_s = qkvpool.tile([128, KT, HPC * DH], DT, name="v_s")

        wq_s = wpool.tile([128, DTILES, HPC * DH], DT, name="wq_s")
        wk_s = wpool.tile([128, DTILES, HPC * DH], DT, name="wk_s")
        wv_s = wpool.tile([128, DTILES, HPC * DH], DT, name="wv_s")
        wo_s = wq_s  # o_proj weights overwrite Wq after the last Q sweep

        def _chunk(dst, src, i, eng=None):
            t0 = i * CSTEP
            (eng or nc.sync).dma_start(dst[:, t0:t0 + CSTEP, :],
                                       src[:, t0:t0 + CSTEP, :])

        x_tiles = {}

        def x_prefetch(sb):
            x_s = xpool.tile([128, DTILES, SBLK], DT, tag="x", name="x_s")
            for i in range(DMA_SPLIT):
                t0 = i * CSTEP
                nc.sync.dma_start(
                    x_s[:, t0:t0 + CSTEP, :],
                    xP[:, sb * DTILES + t0:sb * DTILES + t0 + CSTEP, :])
            x_tiles[sb] = x_s

        # Input loads spread across three DMA queues (sync, scalar, gpsimd):
        # early DMA throughput is descriptor-rate limited per queue, and the
        # Q sweep needs x+Wq as soon as possible after kernel start. The wk
        # triggers go on the gpsimd queue BEFORE the warm-up collective so
        # they are not stuck behind its completion wait.
        x_prefetch(0)
        for i in range(DMA_SPLIT):
            _chunk(wq_s, wqP, i, nc.scalar)
            _chunk(wk_s, wkP, i, nc.gpsimd)
        for i in range(DMA_SPLIT):
            _chunk(wv_s, wvP, i)
        nc.scalar.dma_start(cos_s[:], cosT[:, :])
        nc.scalar.dma_start(srot_s[:], srotT[:, :])
        nc.scalar.dma_start(mask_s[:], masksP[:, :, :])
        # Tiny warm-up AllGather: the first collective of an execution pays
        # ~40us of one-time overhead; absorb it during the projection phase.
        nc.gpsimd.dma_start(wup_in[:, :], ones_b[:, 0:4])
        nc.gpsimd.collective_compute(
            "AllGather", mybir.AluOpType.bypass, replica_groups=RG,
            ins=[wup_in[:, :].opt()], outs=[wup_out[:, :].opt()],
        )

        def rope(ps, dst, s0):
            # RoPE: out = raw*cos + rot(raw)*srot (partition dim = dh)
            cos_b = cos_s[:, s0:s0 + SBLK].unsqueeze(1).broadcast_to(
                [128, HPC, SBLK])
            srot_b = srot_s[:, s0:s0 + SBLK].unsqueeze(1).broadcast_to(
                [128, HPC, SBLK])
            raw = rpool.tile([128, HPC, SBLK], DT, tag="raw", name="raw")
            # split across scalar+vector so the PSUM bank frees fast
            nc.scalar.copy(raw[:, 0:2, :], ps[:, 0:2, :])
            nc.vector.tensor_copy(raw[:, 2:4, :], ps[:, 2:4, :])
            # rotate-half along partitions: engines can't shift partitions,
            # DMA can.
            rot = rpool.tile([128, HPC, SBLK], DT, tag="rot", name="rot")
            nc.scalar.dma_start(rot[0:64], raw[64:128])
            nc.scalar.dma_start(rot[64:128], raw[0:64])
            nc.vector.tensor_mul(rot[:], rot[:], srot_b)
            nc.vector.tensor_mul(raw[:], raw[:], cos_b)
            nc.vector.tensor_add(dst[:, :, s0:s0 + SBLK], raw[:], rot[:])

        # ---------------- projections + RoPE ----------------
        with tc.tile_pool(name="pp", bufs=2, space="PSUM") as ppool:
            for sb in range(NSB):
                s0 = sb * SBLK
                if sb + 1 < NSB:
                    x_prefetch(sb + 1)
                x_s = x_tiles[sb]
                psq = ppool.tile([128, HPC, SBLK], F32, tag="ps", name="psq")
                for dt_i in range(DTILES):
                    st_ = dt_i == 0
                    sp_ = dt_i == DTILES - 1
                    for h in range(HPC):
                        nc.tensor.matmul(
                            psq[:, h, :],
                            lhsT=wq_s[:, dt_i, h * DH:(h + 1) * DH],
                            rhs=x_s[:, dt_i, :],
                            start=st_, stop=sp_,
                        )
                rope(psq, qT, s0)
                if sb == NSB - 1:
                    # overwrite Wq (its last consumer was this block's Q
                    # sweep) with the o_proj weights; lands long before
                    # o_proj starts.
                    for i in range(DMA_SPLIT):
                        _chunk(wq_s, woP, i)
                psk = ppool.tile([128, HPC, SBLK], F32, tag="ps", name="psk")
                for dt_i in range(DTILES):
                    st_ = dt_i == 0
                    sp_ = dt_i == DTILES - 1
                    for h in range(HPC):
                        nc.tensor.matmul(
                            psk[:, h, :],
                            lhsT=wk_s[:, dt_i, h * DH:(h + 1) * DH],
                            rhs=x_s[:, dt_i, :],
                            start=st_, stop=sp_,
                        )
                rope(psk, kTt, s0)
                # V sweep: x^T tiles stationary, W_v moving
                psv = ppool.tile([128, HPC, SBLK], F32, tag="ps", name="psv")
                for dt_i in range(DTILES):
                    st_ = dt_i == 0
                    sp_ = dt_i == DTILES - 1
                    for st in range(4):  # seq sub-tiles of this block
                        nc.tensor.matmul(
                            psv[:, st, :],
                            lhsT=x_s[:, dt_i, st * 128:(st + 1) * 128],
                            rhs=wv_s[:, dt_i, :],
                            start=st_, stop=sp_,
                        )
                for st in range(4):
                    if st < 2:
                        nc.scalar.copy(v_s[:, sb * 4 + st, :], psv[:, st, :])
                    else:
                        nc.vector.tensor_copy(v_s[:, sb * 4 + st, :],
                                              psv[:, st, :])

        # ---------------- attention + AG, then o_proj ----------------
        with tc.tile_pool(name="aps", bufs=2, space="PSUM") as apsum:
            pending_fin = [None]

            def attn_head(sqb, h):
                sq0 = sqb * SBLK
                nkt = 4 * (sqb + 1)
                npair = nkt // 2
                ps_av = apsum.tile([128, SBLK], F32, tag="av", name="ps_av")
                qsum = apool.tile([128, SBLK], DT, tag="qsum", name="qsum",
                                  bufs=2)

                def emit_av(p, st_e):
                    for i in range(2):
                        kt = 2 * p + i
                        nc.tensor.matmul(
                            ps_av[:],
                            lhsT=v_s[:, kt, h * DH:(h + 1) * DH],
                            rhs=st_e[:, i, :],
                            start=(kt == 0), stop=(kt == nkt - 1),
                        )

                # Software-pipelined over score pairs (see module docstring).
                prev = None
                for p in range(npair):
                    ps_st = apsum.tile([128, 2, SBLK], F32, tag="st",
                                       name="ps_st", bufs=2)
                    for i in range(2):
                        kt = 2 * p + i
                        nc.tensor.matmul(
                            ps_st[:, i, :],
                            lhsT=kTt[:, h, kt * 128:(kt + 1) * 128],
                            rhs=qT[:, h, sq0:sq0 + SBLK],
                            start=True, stop=True,
                        )
                    if p == 1 and pending_fin[0] is not None:
                        # previous head's denominator matmul + rescale,
                        # deferred here so the PE does not wait on that
                        # head's trailing exp -> DVE accumulation chain.
                        pending_fin[0]()
                        pending_fin[0] = None
                    st_e = apool.tile([128, 2, SBLK], DT, tag="ste",
                                      name="st_e", bufs=3)
                    nc.scalar.activation(st_e[:], ps_st[:], AF.Exp,
                                         scale=SCALE)
                    pm = 2 * p - (nkt - 4)
                    if pm >= 0:  # diagonal pair: causal 0/1 mask
                        nc.vector.tensor_mul(st_e[:], st_e[:],
                                             mask_s[:, pm:pm + 2, :])
                    # softmax denominator: accumulate exp tiles on the DVE
                    # (bf16), one ones-matmul at the end reduces over
                    # partitions.
                    if p == 0:
                        nc.vector.tensor_add(qsum[:], st_e[:, 0, :],
                                             st_e[:, 1, :])
                    else:
                        nc.vector.tensor_add(qsum[:], qsum[:], st_e[:, 0, :])
                        nc.vector.tensor_add(qsum[:], qsum[:], st_e[:, 1, :])
                    if prev is not None:
                        emit_av(p - 1, prev)
                    prev = st_e
                emit_av(npair - 1, prev)

                def fin():
                    ps_den = apsum.tile([128, SBLK], F32, tag="dn",
                                        name="ps_den", bufs=2)
                    nc.tensor.matmul(ps_den[:], lhsT=ones_b[:], rhs=qsum[:],
                                     start=True, stop=True)
                    rden = apool.tile([128, SBLK], F32, tag="rden",
                                      name="rden", bufs=2)
                    nc.vector.reciprocal_approx_fast(rden[:], ps_den[:])
                    vout = apool.tile([128, SBLK], DT, tag="vout",
                                      name="vout", bufs=4)
                    nc.vector.tensor_mul(vout[:], ps_av[:], rden[:])
                    nc.sync.dma_start(
                        vloc[sqb][h * DH:(h + 1) * DH, :], vout[:])
                pending_fin[0] = fin

            def attn_block(sqb):
                for h in range(HPC):
                    attn_head(sqb, h)
                # flush the last head's finisher before a following
                # AllGather is emitted (its vloc write must precede the
                # collective in program order for dependency tracking).
                pending_fin[0]()
                pending_fin[0] = None

            vg_tiles = []

            def ag_block(b):
                # Collective only - interleaving the gathered-vals loads on
                # the gpsimd queue (an earlier version) delayed the next
                # doorbell and made every AllGather 1.5-2x slower.
                nc.gpsimd.collective_compute(
                    "AllGather", mybir.AluOpType.bypass, replica_groups=RG,
                    ins=[vloc[b][:, :].opt()], outs=[vgath[b][:, :].opt()],
                )

            def vg_load(b):
                # Post-attention, the sync engine/queue is idle: its blocking
                # wait on AllGather b's completion delays nothing else.
                vg = vgpool.tile([128, DTILES, SBLK], DT, tag="vg", name="vg")
                for i_ in range(2):
                    t0 = i_ * (DTILES // 2)
                    nc.sync.dma_start(
                        vg[:, t0:t0 + DTILES // 2, :],
                        vgath[b][t0 * 128:(t0 + DTILES // 2) * 128, :]
                        .rearrange("(t p) s -> p t s", p=128))
                vg_tiles.append(vg)

            def oproj_block(b):
                s0 = b * SBLK
                vg = vg_tiles[b]
                for ct in range(OCOLS // 128):
                    ps_o = apsum.tile([128, SBLK], F32, tag="dn", name="ps_o",
                                      bufs=2)
                    for dt_i in range(DTILES):
                        nc.tensor.matmul(
                            ps_o[:],
                            lhsT=wo_s[:, dt_i, ct * 128:(ct + 1) * 128],
                            rhs=vg[:, dt_i, :],
                            start=(dt_i == 0), stop=(dt_i == DTILES - 1),
                        )
                    ob = obpool.tile([128, SBLK], F32, tag="ob", name="ob")
                    nc.scalar.copy(ob[:], ps_o[:])
                    nc.scalar.dma_start(
                        outT[ct * 128:(ct + 1) * 128, s0:s0 + SBLK], ob[:])

            for sqb in range(NSB):
                attn_block(sqb)
                ag_block(sqb)
            for c in range(NSB):
                # the wait hint keeps the scheduler from hoisting these
                # blocking-wait loads into the attention section of the sync
                # engine stream (which would stall attention's vout DMAs
                # behind an AllGather completion)
                with tc.tile_wait_until(0.42 + 0.002 * c):
                    vg_load(c)
            for c in range(NSB):
                with tc.tile_wait_until(0.45 + 0.01 * c):
                    oproj_block(c)


_NC_CACHE = None


def build_program():
    global _NC_CACHE
    if _NC_CACHE is not None:
        return _NC_CACHE
    nc = bacc.Bacc("TRN2", target_bir_lowering=False, debug=False,
                   enable_asserts=False, num_devices=NCORES)
    with tile.TileContext(nc) as tc:
        _emit(tc)
    nc.compile()
    _NC_CACHE = nc
    return nc


def _pack_pmaj(a2d):
    """[T*128, C] row-major -> [128, T, C] with per-partition-contiguous
    lines (partition p holds rows p, 128+p, ... consecutively)."""
    t = a2d.shape[0] // 128
    return np.ascontiguousarray(
        a2d.reshape(t, 128, a2d.shape[1]).transpose(1, 0, 2)).astype(NPDT)


def _prep_inputs(x, cos, sin, Wq, Wk, Wv, Wo):
    """Build the 8 per-core input maps (host-side sharding only)."""
    x = np.asarray(x, dtype=np.float32)
    cos = np.asarray(cos, dtype=np.float32)
    sin = np.asarray(sin, dtype=np.float32)
    Wq = np.asarray(Wq, dtype=np.float32)
    Wk = np.asarray(Wk, dtype=np.float32)
    Wv = np.asarray(Wv, dtype=np.float32)
    Wo = np.asarray(Wo, dtype=np.float32)

    cosT = np.ascontiguousarray(cos.T).astype(NPDT)             # [128, S]
    sinT = np.ascontiguousarray(sin.T)
    srotT = np.concatenate([-sinT[:64], sinT[64:]], axis=0).astype(NPDT)

    iota = np.arange(SBLK)[None, :]
    rows = np.arange(128)[:, None]
    masks = np.stack(
        [(128 * p + rows <= iota) for p in range(4)]).astype(NPDT)  # [4,128,512]
    masksP = np.ascontiguousarray(masks.transpose(1, 0, 2))         # [128,4,512]

    # x packed [128, NSB*DTILES, SBLK]: element [p, sb*16+t, s] =
    # x[g][sb*512+s, t*128+p], per-(p,sb) line 16KB contiguous.
    xPg = [np.ascontiguousarray(
        x[g].T.reshape(DTILES, 128, NSB, SBLK).transpose(1, 2, 0, 3)
        .reshape(128, NSB * DTILES, SBLK)).astype(NPDT) for g in range(B)]

    in_maps = []
    for c in range(NCORES):
        g, r = c // TP, c % TP
        hs = slice(r * HPC * DH, (r + 1) * HPC * DH)
        in_maps.append({
            "xP": xPg[g],
            "wqP": _pack_pmaj(Wq[hs].T),
            "wkP": _pack_pmaj(Wk[hs].T),
            "wvP": _pack_pmaj(Wv[hs].T),
            "woP": _pack_pmaj(Wo[r * OCOLS:(r + 1) * OCOLS].T),
            "cosT": cosT,
            "srotT": srotT,
            "masksP": masksP,
        })
    return in_maps


def run(inputs, trace=False, trace_cores=None):
    nc = build_program()
    in_maps = _prep_inputs(**inputs)
    res = run_bass_kernel_spmd(
        nc, in_maps, core_ids=list(range(NCORES)),
        trace=trace, trace_cores=trace_cores,
    )
    out = np.empty((B, S, D), dtype=np.float32)
    for c in range(NCORES):
        g, r = c // TP, c % TP
        out[g, :, r * OCOLS:(r + 1) * OCOLS] = res.results[c]["outT"].T
    return out, res


def kernel(**inputs):
    out, _ = run(inputs)
    return out


# revision 22
# speedup vs baseline: 1.0984x; 1.0201x over previous
"""Llama attention layer (B=2, S=2048, D=2048, H=16, DH=128) on 8 TRN2 NeuronCores.

Sharding: 2-way data parallel over batch x 4-way tensor parallel over heads.
Core c: batch g = c // 4, heads 4r..4r+3 where r = c % 4.
Projections are column-parallel (each core computes Q/K/V for its 4 heads),
attention is fully local per (batch, head), then the per-head attention
outputs (kept transposed, [dim, seq]) are AllGather'd within each 4-core
batch group, and o_proj is column-parallel: core c computes output columns
r*512..(r+1)*512 of its batch. Host concatenates - no host-side compute.

Schedule: attention blocks run in ascending order; their outputs are
gathered with just TWO AllGathers (blocks {0,1} and {2,3}) because
collectives serialize on the issuing engine - with four gathers the chain
(25-45us per op, straggler-bound) finished ~40us after the last o_proj
consumer needed it. AG{01} fires at attn1-end and AG{23} at attn3-end, both
input-bound; o_proj runs at the end of the PE stream consuming blocks in
order 0..3, so only o_proj2 can briefly wait. All inputs are packed
partition-major on the host so DMA lines are 4KB+, spread over three DMA
queues (early DMA is descriptor-rate limited per queue).

All matmul operands are bf16 (fp32 accumulation in PSUM); softmax runs
without max-subtraction (scores are O(6), exp is safe in fp32); the
denominator is accumulated on the vector engine in bf16 and reduced over
partitions with a single ones-matmul per (head, seq-block), deferred into
the next head's stream so the PE never waits on the trailing exp chain. The
attention inner loop is software-pipelined: pair p+1's QK matmuls are
emitted before pair p's AV matmuls, hiding the scalar-engine exp latency.
"""

import os
import sys

for _p in ("/opt/trn_rl_repo", "/root/.axon_site/_ro/trn_rl_repo"):
    if os.path.isdir(_p) and _p not in sys.path:
        sys.path.append(_p)

import numpy as np
import ml_dtypes

import concourse.bass as bass
import concourse.tile as tile
import concourse.mybir as mybir
from concourse import bacc
from concourse.bass_utils import run_bass_kernel_spmd

F32 = mybir.dt.float32
BF16 = mybir.dt.bfloat16
AF = mybir.ActivationFunctionType

B, S, D, H, DH = 2, 2048, 2048, 16, 128
NCORES = 8
TP = 4                 # cores per batch group
HPC = H // TP          # heads per core = 4
SBLK = 512             # seq block (matmul moving size)
NSB = S // SBLK        # 4
DTILES = D // 128      # 16 contraction tiles
KT = S // 128          # 16 key tiles
OCOLS = D // TP        # 512 output columns per core
SCALE = 1.0 / float(np.sqrt(DH))
DMA_SPLIT = 4          # split big input DMAs so compute starts early
CSTEP = DTILES // DMA_SPLIT
RG = [[0, 1, 2, 3], [4, 5, 6, 7]]

DT = BF16              # matmul operand dtype
NPDT = ml_dtypes.bfloat16


def _emit(tc):
    nc = tc.nc
    # Inputs packed partition-major on host: per partition line contiguous.
    xP = nc.dram_tensor("xP", [128, NSB * DTILES, SBLK], DT,
                        kind="ExternalInput").ap()
    wqP = nc.dram_tensor("wqP", [128, DTILES, HPC * DH], DT,
                         kind="ExternalInput").ap()
    wkP = nc.dram_tensor("wkP", [128, DTILES, HPC * DH], DT,
                         kind="ExternalInput").ap()
    wvP = nc.dram_tensor("wvP", [128, DTILES, HPC * DH], DT,
                         kind="ExternalInput").ap()
    woP = nc.dram_tensor("woP", [128, DTILES, OCOLS], DT,
                         kind="ExternalInput").ap()
    cosT = nc.dram_tensor("cosT", [DH, S], DT, kind="ExternalInput").ap()
    srotT = nc.dram_tensor("srotT", [DH, S], DT, kind="ExternalInput").ap()
    masksP = nc.dram_tensor("masksP", [128, 4, SBLK], DT,
                            kind="ExternalInput").ap()
    outT = nc.dram_tensor("outT", [OCOLS, S], F32, kind="ExternalOutput").ap()

    # One AllGather bounce pair per 512-seq block.
    vloc = [nc.dram_tensor(f"vals_loc_{b}", [HPC * DH, SBLK], DT).ap()
            for b in range(NSB)]
    vgath = [nc.dram_tensor(f"vals_gath_{b}", [D, SBLK], DT).ap()
             for b in range(NSB)]
    wup_in = nc.dram_tensor("wup_in", [128, 4], DT).ap()
    wup_out = nc.dram_tensor("wup_out", [512, 4], DT).ap()

    with tc.tile_pool(name="const", bufs=1) as cpool, \
         tc.tile_pool(name="qkv", bufs=1) as qkvpool, \
         tc.tile_pool(name="w", bufs=1) as wpool, \
         tc.tile_pool(name="xs", bufs=2) as xpool, \
         tc.tile_pool(name="rope", bufs=1) as rpool, \
         tc.tile_pool(name="att", bufs=3) as apool, \
         tc.tile_pool(name="vg", bufs=2) as vgpool, \
         tc.tile_pool(name="ob", bufs=2) as obpool:

        cos_s = cpool.tile([128, S], DT, name="cos_s")
        srot_s = cpool.tile([128, S], DT, name="srot_s")
        mask_s = cpool.tile([128, 4, SBLK], DT, name="mask_s")
        # bf16 ones matrix: ones.T @ x sums x over partitions and yields the
        # result replicated across all 128 partitions (DVE cannot broadcast
        # along partitions, so produce the softmax denominator pre-broadcast).
        ones_b = cpool.tile([128, 128], DT, name="ones_b")
        nc.vector.memset(ones_b[:], 1.0)

        qT = qkvpool.tile([128, HPC, S], DT, name="qT")
        kTt = qkvpool.tile([128, HPC, S], DT, name="kTt")
        v_s = qkvpool.tile([128, KT, HPC * DH], DT, name="v_s")

        wq_s = wpool.tile([128, DTILES, HPC * DH], DT, name="wq_s")
        wk_s = wpool.tile([128, DTILES, HPC * DH], DT, name="wk_s")
        wv_s = wpool.tile([128, DTILES, HPC * DH], DT, name="wv_s")
        wo_s = wq_s  # o_proj weights overwrite Wq after the last Q sweep

        def _chunk(dst, src, i, eng=None):
            t0 = i * CSTEP
            (eng or nc.sync).dma_start(dst[:, t0:t0 + CSTEP, :],
                                       src[:, t0:t0 + CSTEP, :])

        x_tiles = {}

        def x_prefetch(sb):
            x_s = xpool.tile([128, DTILES, SBLK], DT, tag="x", name="x_s")
            for i in range(DMA_SPLIT):
                t0 = i * CSTEP
                nc.sync.dma_start(
                    x_s[:, t0:t0 + CSTEP, :],
                    xP[:, sb * DTILES + t0:sb * DTILES + t0 + CSTEP, :])
            x_tiles[sb] = x_s

        # Input loads spread across three DMA queues (sync, scalar, gpsimd):
        # early DMA throughput is descriptor-rate limited per queue, and the
        # Q sweep needs x+Wq as soon as possible after kernel start. The wk
        # triggers go on the gpsimd queue BEFORE the warm-up collective so
        # they are not stuck behind its completion wait.
        x_prefetch(0)
        for i in range(DMA_SPLIT):
            _chunk(wq_s, wqP, i, nc.scalar)
            _chunk(wk_s, wkP, i, nc.gpsimd)
        for i in range(DMA_SPLIT):
            _chunk(wv_s, wvP, i)
        nc.scalar.dma_start(cos_s[:], cosT[:, :])
        nc.scalar.dma_start(srot_s[:], srotT[:, :])
        nc.scalar.dma_start(mask_s[:], masksP[:, :, :])
        # Tiny warm-up AllGather: the first collective of an execution pays
        # ~40us of one-time overhead; absorb it during the projection phase.
        nc.gpsimd.dma_start(wup_in[:, :], ones_b[:, 0:4])
        nc.gpsimd.collective_compute(
            "AllGather", mybir.AluOpType.bypass, replica_groups=RG,
            ins=[wup_in[:, :].opt()], outs=[wup_out[:, :].opt()],
        )

        def rope(ps, dst, s0):
            # RoPE: out = raw*cos + rot(raw)*srot (partition dim = dh)
            cos_b = cos_s[:, s0:s0 + SBLK].unsqueeze(1).broadcast_to(
                [128, HPC, SBLK])
            srot_b = srot_s[:, s0:s0 + SBLK].unsqueeze(1).broadcast_to(
                [128, HPC, SBLK])
            raw = rpool.tile([128, HPC, SBLK], DT, tag="raw", name="raw")
            # split across scalar+vector so the PSUM bank frees fast
            nc.scalar.copy(raw[:, 0:2, :], ps[:, 0:2, :])
            nc.vector.tensor_copy(raw[:, 2:4, :], ps[:, 2:4, :])
            # rotate-half along partitions: engines can't shift partitions,
            # DMA can.
            rot = rpool.tile([128, HPC, SBLK], DT, tag="rot", name="rot")
            nc.scalar.dma_start(rot[0:64], raw[64:128])
            nc.scalar.dma_start(rot[64:128], raw[0:64])
            nc.vector.tensor_mul(rot[:], rot[:], srot_b)
            nc.vector.tensor_mul(raw[:], raw[:], cos_b)
            nc.vector.tensor_add(dst[:, :, s0:s0 + SBLK], raw[:], rot[:])

        # ---------------- projections + RoPE ----------------
        with tc.tile_pool(name="pp", bufs=2, space="PSUM") as ppool:
            for sb in range(NSB):
                s0 = sb * SBLK
                if sb + 1 < NSB:
                    x_prefetch(sb + 1)
                x_s = x_tiles[sb]
                psq = ppool.tile([128, HPC, SBLK], F32, tag="ps", name="psq")
                for dt_i in range(DTILES):
                    st_ = dt_i == 0
                    sp_ = dt_i == DTILES - 1
                    for h in range(HPC):
                        nc.tensor.matmul(
                            psq[:, h, :],
                            lhsT=wq_s[:, dt_i, h * DH:(h + 1) * DH],
                            rhs=x_s[:, dt_i, :],
                            start=st_, stop=sp_,
                        )
                rope(psq, qT, s0)
                if sb == NSB - 1:
                    # overwrite Wq (its last consumer was this block's Q
                    # sweep) with the o_proj weights; lands long before
                    # o_proj starts.
                    for i in range(DMA_SPLIT):
                        _chunk(wq_s, woP, i)
                psk = ppool.tile([128, HPC, SBLK], F32, tag="ps", name="psk")
                for dt_i in range(DTILES):
                    st_ = dt_i == 0
                    sp_ = dt_i == DTILES - 1
                    for h in range(HPC):
                        nc.tensor.matmul(
                            psk[:, h, :],
                            lhsT=wk_s[:, dt_i, h * DH:(h + 1) * DH],
                            rhs=x_s[:, dt_i, :],
                            start=st_, stop=sp_,
                        )
                rope(psk, kTt, s0)
                # V sweep: x^T tiles stationary, W_v moving
                psv = ppool.tile([128, HPC, SBLK], F32, tag="ps", name="psv")
                for dt_i in range(DTILES):
                    st_ = dt_i == 0
                    sp_ = dt_i == DTILES - 1
                    for st in range(4):  # seq sub-tiles of this block
                        nc.tensor.matmul(
                            psv[:, st, :],
                            lhsT=x_s[:, dt_i, st * 128:(st + 1) * 128],
                            rhs=wv_s[:, dt_i, :],
                            start=st_, stop=sp_,
                        )
                for st in range(4):
                    if st < 2:
                        nc.scalar.copy(v_s[:, sb * 4 + st, :], psv[:, st, :])
                    else:
                        nc.vector.tensor_copy(v_s[:, sb * 4 + st, :],
                                              psv[:, st, :])

        # ---------------- attention + AG, then o_proj ----------------
        with tc.tile_pool(name="aps", bufs=2, space="PSUM") as apsum:
            pending_fin = [None]

            def attn_head(sqb, h):
                sq0 = sqb * SBLK
                nkt = 4 * (sqb + 1)
                npair = nkt // 2
                ps_av = apsum.tile([128, SBLK], F32, tag="av", name="ps_av",
                                   bufs=1)
                qsum = apool.tile([128, SBLK], DT, tag="qsum", name="qsum",
                                  bufs=2)

                def emit_av(p, st_e):
                    for i in range(2):
                        kt = 2 * p + i
                        nc.tensor.matmul(
                            ps_av[:],
                            lhsT=v_s[:, kt, h * DH:(h + 1) * DH],
                            rhs=st_e[:, i, :],
                            start=(kt == 0), stop=(kt == nkt - 1),
                        )

                # Software-pipelined over score pairs with a lag of TWO:
                # pair p's AV matmuls are emitted after pair p+2's QK
                # matmuls, so each exp has two QK pairs (~0.9us) of PE work
                # covering its latency - a lag of one (432ns) left every
                # pair stalling ~0.4us on the scalar engine.
                pend = []
                for p in range(npair):
                    ps_st = apsum.tile([128, 2, SBLK], F32, tag="st",
                                       name="ps_st", bufs=3)
                    for i in range(2):
                        kt = 2 * p + i
                        nc.tensor.matmul(
                            ps_st[:, i, :],
                            lhsT=kTt[:, h, kt * 128:(kt + 1) * 128],
                            rhs=qT[:, h, sq0:sq0 + SBLK],
                            start=True, stop=True,
                        )
                    if p == 1 and pending_fin[0] is not None:
                        # previous head's denominator matmul + rescale,
                        # deferred here so the PE does not wait on that
                        # head's trailing exp -> DVE accumulation chain.
                        pending_fin[0]()
                        pending_fin[0] = None
                    st_e = apool.tile([128, 2, SBLK], DT, tag="ste",
                                      name="st_e", bufs=4)
                    nc.scalar.activation(st_e[:], ps_st[:], AF.Exp,
                                         scale=SCALE)
                    pm = 2 * p - (nkt - 4)
                    if pm >= 0:  # diagonal pair: causal 0/1 mask
                        nc.vector.tensor_mul(st_e[:], st_e[:],
                                             mask_s[:, pm:pm + 2, :])
                    # softmax denominator: accumulate exp tiles on the DVE
                    # (bf16), one ones-matmul at the end reduces over
                    # partitions.
                    if p == 0:
                        nc.vector.tensor_add(qsum[:], st_e[:, 0, :],
                                             st_e[:, 1, :])
                    else:
                        nc.vector.tensor_add(qsum[:], qsum[:], st_e[:, 0, :])
                        nc.vector.tensor_add(qsum[:], qsum[:], st_e[:, 1, :])
                    pend.append(st_e)
                    if p >= 2:
                        emit_av(p - 2, pend[p - 2])
                for p in range(max(0, npair - 2), npair):
                    emit_av(p, pend[p])

                def fin():
                    ps_den = apsum.tile([128, SBLK], F32, tag="dn",
                                        name="ps_den", bufs=1)
                    nc.tensor.matmul(ps_den[:], lhsT=ones_b[:], rhs=qsum[:],
                                     start=True, stop=True)
                    rden = apool.tile([128, SBLK], F32, tag="rden",
                                      name="rden", bufs=2)
                    nc.vector.reciprocal_approx_fast(rden[:], ps_den[:])
                    vout = apool.tile([128, SBLK], DT, tag="vout",
                                      name="vout", bufs=4)
                    nc.vector.tensor_mul(vout[:], ps_av[:], rden[:])
                    nc.sync.dma_start(
                        vloc[sqb][h * DH:(h + 1) * DH, :], vout[:])
                pending_fin[0] = fin

            def attn_block(sqb):
                for h in range(HPC):
                    attn_head(sqb, h)
                # flush the last head's finisher before a following
                # AllGather is emitted (its vloc write must precede the
                # collective in program order for dependency tracking).
                pending_fin[0]()
                pending_fin[0] = None

            vg_tiles = []

            def ag_block(b):
                # Collective only - interleaving the gathered-vals loads on
                # the gpsimd queue (an earlier version) delayed the next
                # doorbell and made every AllGather 1.5-2x slower.
                nc.gpsimd.collective_compute(
                    "AllGather", mybir.AluOpType.bypass, replica_groups=RG,
                    ins=[vloc[b][:, :].opt()], outs=[vgath[b][:, :].opt()],
                )

            def vg_load(b):
                # Post-attention, the sync engine/queue is idle: its blocking
                # wait on AllGather b's completion delays nothing else.
                vg = vgpool.tile([128, DTILES, SBLK], DT, tag="vg", name="vg")
                for i_ in range(2):
                    t0 = i_ * (DTILES // 2)
                    nc.sync.dma_start(
                        vg[:, t0:t0 + DTILES // 2, :],
                        vgath[b][t0 * 128:(t0 + DTILES // 2) * 128, :]
                        .rearrange("(t p) s -> p t s", p=128))
                vg_tiles.append(vg)

            def oproj_block(b):
                s0 = b * SBLK
                vg = vg_tiles[b]
                for ct in range(OCOLS // 128):
                    # alternate between the two 1-bank tags for double
                    # buffering (attention is over by o_proj time)
                    ps_o = apsum.tile([128, SBLK], F32,
                                      tag=("dn" if ct % 2 == 0 else "av"),
                                      name="ps_o", bufs=1)
                    for dt_i in range(DTILES):
                        nc.tensor.matmul(
                            ps_o[:],
                            lhsT=wo_s[:, dt_i, ct * 128:(ct + 1) * 128],
                            rhs=vg[:, dt_i, :],
                            start=(dt_i == 0), stop=(dt_i == DTILES - 1),
                        )
                    ob = obpool.tile([128, SBLK], F32, tag="ob", name="ob")
                    nc.scalar.copy(ob[:], ps_o[:])
                    nc.scalar.dma_start(
                        outT[ct * 128:(ct + 1) * 128, s0:s0 + SBLK], ob[:])

            for sqb in range(NSB):
                attn_block(sqb)
                ag_block(sqb)
            for c in range(NSB):
                # the wait hint keeps the scheduler from hoisting these
                # blocking-wait loads into the attention section of the sync
                # engine stream (which would stall attention's vout DMAs
                # behind an AllGather completion)
                with tc.tile_wait_until(0.42 + 0.002 * c):
                    vg_load(c)
            for c in range(NSB):
                with tc.tile_wait_until(0.45 + 0.01 * c):
                    oproj_block(c)


_NC_CACHE = None


def build_program():
    global _NC_CACHE
    if _NC_CACHE is not None:
        return _NC_CACHE
    nc = bacc.Bacc("TRN2", target_bir_lowering=False, debug=False,
                   enable_asserts=False, num_devices=NCORES)
    with tile.TileContext(nc) as tc:
        _emit(tc)
    nc.compile()
    _NC_CACHE = nc
    return nc


def _pack_pmaj(a2d):
    """[T*128, C] row-major -> [128, T, C] with per-partition-contiguous
    lines (partition p holds rows p, 128+p, ... consecutively)."""
    t = a2d.shape[0] // 128
    return np.ascontiguousarray(
        a2d.reshape(t, 128, a2d.shape[1]).transpose(1, 0, 2)).astype(NPDT)


def _prep_inputs(x, cos, sin, Wq, Wk, Wv, Wo):
    """Build the 8 per-core input maps (host-side sharding only)."""
    x = np.asarray(x, dtype=np.float32)
    cos = np.asarray(cos, dtype=np.float32)
    sin = np.asarray(sin, dtype=np.float32)
    Wq = np.asarray(Wq, dtype=np.float32)
    Wk = np.asarray(Wk, dtype=np.float32)
    Wv = np.asarray(Wv, dtype=np.float32)
    Wo = np.asarray(Wo, dtype=np.float32)

    cosT = np.ascontiguousarray(cos.T).astype(NPDT)             # [128, S]
    sinT = np.ascontiguousarray(sin.T)
    srotT = np.concatenate([-sinT[:64], sinT[64:]], axis=0).astype(NPDT)

    iota = np.arange(SBLK)[None, :]
    rows = np.arange(128)[:, None]
    masks = np.stack(
        [(128 * p + rows <= iota) for p in range(4)]).astype(NPDT)  # [4,128,512]
    masksP = np.ascontiguousarray(masks.transpose(1, 0, 2))         # [128,4,512]

    # x packed [128, NSB*DTILES, SBLK]: element [p, sb*16+t, s] =
    # x[g][sb*512+s, t*128+p], per-(p,sb) line 16KB contiguous.
    xPg = [np.ascontiguousarray(
        x[g].T.reshape(DTILES, 128, NSB, SBLK).transpose(1, 2, 0, 3)
        .reshape(128, NSB * DTILES, SBLK)).astype(NPDT) for g in range(B)]

    in_maps = []
    for c in range(NCORES):
        g, r = c // TP, c % TP
        hs = slice(r * HPC * DH, (r + 1) * HPC * DH)
        in_maps.append({
            "xP": xPg[g],
            "wqP": _pack_pmaj(Wq[hs].T),
            "wkP": _pack_pmaj(Wk[hs].T),
            "wvP": _pack_pmaj(Wv[hs].T),
            "woP": _pack_pmaj(Wo[r * OCOLS:(r + 1) * OCOLS].T),
            "cosT": cosT,
            "srotT": srotT,
            "masksP": masksP,
        })
    return in_maps


def run(inputs, trace=False, trace_cores=None):
    nc = build_program()
    in_maps = _prep_inputs(**inputs)
    res = run_bass_kernel_spmd(
        nc, in_maps, core_ids=list(range(NCORES)),
        trace=trace, trace_cores=trace_cores,
    )
    out = np.empty((B, S, D), dtype=np.float32)
    for c in range(NCORES):
        g, r = c // TP, c % TP
        out[g, :, r * OCOLS:(r + 1) * OCOLS] = res.results[c]["outT"].T
    return out, res


def kernel(**inputs):
    out, _ = run(inputs)
    return out
